# revision 48
# baseline (speedup 1.0000x reference)
"""Multi-head self-attention on 8 Trainium2 NeuronCores.

Sharding: core c = b*4 + g handles batch b (of 2) and head-group g (4 heads
of 16). Per core: full qkv projection for its 4 heads, attention, and a
partial output projection (row-slice of Wout). Host sums the 4 partials per
batch and adds bout.

The ACT engine is the hard floor (16.8M softmax exps at 1 elem/cycle/lane =
~109us/core), so the kernel keeps ACT fed continuously: only pair-0's
projection runs up front; pair-1's projection matmuls drip between pair-0
attention jobs inside the ACT slack. All matmul operands are bf16 so weight
loads stream via LDWEIGHTS (fp32r stationaries force serial self-loading
matmuls on HW). Scores for the two heads of a pair are issued as PE row
tiles (0,0)/(64,0): contraction is head_dim=64, so both 64-row tiles execute
concurrently, doubling score throughput. One [128,1024] exp per job covers
both heads. PV keeps the ones-column denominator trick; normalization is
fused into the PSUM evacuation (tensor_mul with the broadcast reciprocal).
"""

import os
from contextlib import ExitStack

import ml_dtypes
import numpy as np

import concourse.bass as bass
import concourse.bacc as bacc
import concourse.tile as tile
from concourse import mybir
from concourse._compat import with_exitstack
from concourse.bass_utils import run_bass_kernel_spmd

B, S, E, H = 2, 2048, 1024, 16
HD = 64
SCALE = HD ** -0.5
NCORES = 8
GROUPS = 4                 # head-groups per batch == cores per batch
HPG = H // GROUPS          # 4 heads per core
DG = HPG * HD              # 256 qkv cols per core per projection
KC = E // 128              # 8 contraction chunks
NT = S // 512              # 4 q/token chunks of 512
SKT = S // 128             # 16 key tiles of 128
VBLK = 65                  # v block cols: 64 v dims + ones column
DEPTH = 4                  # attention software-pipeline depth
DRIP_NS = 460.0            # PE-ns of dripped projection work per attention job

FP = mybir.dt.float32
BF = mybir.dt.bfloat16


@with_exitstack
def _mha_body(ctx: ExitStack, tc: tile.TileContext, xt, w, bqkv, wo, y):
    nc = tc.nc
    main = ctx.enter_context(tc.tile_pool(name="main", bufs=1))

    qT = [main.tile([128, S], BF, name=f"qT{p}") for p in range(2)]
    kT = [main.tile([128, S], BF, name=f"kT{p}") for p in range(2)]
    v_store = main.tile([128, SKT * HPG * VBLK], BF)   # [128, 4160]
    attn = [main.tile([128, S], BF, name=f"attn{p}") for p in range(2)]
    wo_sb = [main.tile([128, E], BF, name=f"wo{p}") for p in range(2)]
    b_sb = main.tile([128, 4], FP)
    vb = main.tile([128, DG], FP)
    den_all = main.tile([1, 16 * 512], FP)
    rden_all = main.tile([1, 16 * 512], FP)
    xts = [main.tile([128, S], BF, name=f"xts{k}") for k in range(KC)]
    wts = [main.tile([128, 768], BF, name=f"wts{k}") for k in range(KC)]

    for m in range(4):
        nc.gpsimd.dma_start(out=b_sb[:, m : m + 1], in_=bqkv[m * 128 : (m + 1) * 128, :])
    vb_row = main.tile([1, DG], FP)
    nc.gpsimd.dma_start(out=vb_row, in_=bqkv[512:768, :])
    nc.gpsimd.partition_broadcast(vb, vb_row)

    # weights first, then x in token-column blocks: the first projection
    # waves need only token block 0, so compute starts ~3MB earlier
    for k in range(KC):
        wq = nc.sync if k % 2 == 0 else nc.gpsimd
        wq.dma_start(out=wts[k], in_=w[k * 128 : (k + 1) * 128, :])
    for nb in range(NT):
        for k in range(KC):
            xq = nc.default_dma_engine if k % 2 == 0 else nc.scalar
            xq.dma_start(
                out=xts[k][:, nb * 512 : (nb + 1) * 512],
                in_=xt[k * 128 : (k + 1) * 128, nb * 512 : (nb + 1) * 512],
            )
    for p in range(2):
        nc.gpsimd.dma_start(out=wo_sb[p], in_=wo[p * 128 : (p + 1) * 128, :])

    vs_view = v_store.rearrange("p (j c) -> p j c", c=VBLK)
    ones_src = main.tile([128, SKT * HPG], BF)
    nc.vector.memset(ones_src, 1.0)
    nc.vector.tensor_copy(
        vs_view[:, :, 64:65], ones_src.rearrange("p (j c) -> p j c", c=1)
    )
    # dummy exp: pulls the ~2.7us ACT_TABLE_LOAD into the DMA window
    act_warm = main.tile([1, 1], FP)
    nc.scalar.activation(
        act_warm, ones_src[0:1, 0:1], mybir.ActivationFunctionType.Exp, scale=1.0
    )

    # m: 0=qT0 (heads 0,1), 1=qT1, 2=kT0, 3=kT1
    dsts = [qT[0], qT[1], kT[0], kT[1]]
    bstack = ctx.enter_context(ExitStack())
    qk_ps = bstack.enter_context(tc.tile_pool(name="qk_ps", bufs=1, space="PSUM"))
    v_ps = qk_ps

    # projection work units: ("q", m, n) -> one [128,512] q/k block;
    # ("v", pair, tt) -> one [128,128] v token-tile for the pair's 2 heads
    emitted = set()

    def emit_q_wave(m, n, yielding):
        ps = qk_ps.tile([128, 512], FP, name="qps")
        for k in range(KC):
            nc.tensor.matmul(
                ps,
                wts[k][:, m * 128 : (m + 1) * 128],
                xts[k][:, n * 512 : (n + 1) * 512],
                start=(k == 0),
                stop=(k == KC - 1),
            )
            if yielding:
                yield 213.0
        nc.vector.tensor_scalar_add(
            dsts[m][:, n * 512 : (n + 1) * 512], ps, b_sb[:, m : m + 1]
        )
        emitted.add(("q", m, n))
        if yielding:
            yield 0.0

    def emit_v_wave(pair, tt, yielding):
        # shares the qps PSUM slot (only cols 0:128 used) to save a bank
        vp = v_ps.tile([128, 512], FP, name="qps")[:, 0:128]
        for k in range(KC):
            nc.tensor.matmul(
                vp,
                xts[k][:, tt * 128 : (tt + 1) * 128],
                wts[k][:, 512 + pair * 128 : 640 + pair * 128],
                start=(k == 0),
                stop=(k == KC - 1),
            )
            if yielding:
                yield 80.0
        nc.vector.tensor_add(
            vs_view[:, tt * HPG + 2 * pair : tt * HPG + 2 * pair + 2, 0:64],
            vp.rearrange("p (j c) -> p j c", c=64),
            vb[:, pair * 128 : pair * 128 + 128].rearrange("p (j c) -> p j c", c=64),
        )
        emitted.add(("v", pair, tt))
        if yielding:
            yield 0.0

    # ---- phase A0: minimal prefix — kT0 keys 0:512, qT0 block 0, v0 tile 0 ----
    for _ in emit_q_wave(2, 0, False):
        pass
    for _ in emit_q_wave(0, 0, False):
        pass
    for _ in emit_v_wave(0, 0, False):
        pass

    # everything else drips between attention jobs, ordered so each unit
    # lands ahead of its consumer (deadlines in B0/B1 job indices; the
    # drip_until gates force just-in-time emission if the budget lags)
    drip_items = (
        [("q", 2, 1), ("v", 0, 1), ("v", 0, 2), ("v", 0, 3)]
        + [("q", 2, 2), ("v", 0, 4), ("v", 0, 5), ("v", 0, 6)]
        + [("q", 2, 3), ("v", 0, 7), ("v", 0, 8), ("v", 0, 9)]
        + [("q", 0, 1)]
        + [("v", 0, tt) for tt in (10, 11, 12)]
        + [("q", 0, 2)]
        + [("v", 0, tt) for tt in (13, 14, 15)]
        + [("q", 0, 3)]
        + [("q", 3, n) for n in range(NT)]
        + [("q", 1, 0)]
        + [("v", 1, tt) for tt in range(4)]
        + [("q", 1, 1)]
        + [("v", 1, tt) for tt in range(4, 9)]
        + [("q", 1, 2)]
        + [("v", 1, tt) for tt in range(9, 13)]
        + [("q", 1, 3)]
        + [("v", 1, tt) for tt in range(13, 16)]
    )

    def drip_gen():
        for item in drip_items:
            if item[0] == "q":
                yield from emit_q_wave(item[1], item[2], True)
            else:
                yield from emit_v_wave(item[1], item[2], True)

    drip = drip_gen()
    drip_done = False
    drip_budget = 0.0

    def drip_steps(ns):
        nonlocal drip_budget, drip_done
        drip_budget += ns
        while not drip_done and drip_budget > 0:
            try:
                drip_budget -= next(drip)
            except StopIteration:
                drip_done = True

    def drip_until(*keys):
        # program-order deadline: a consumer on the PE queue must be emitted
        # after its producers; force-drain the drip to the needed unit
        nonlocal drip_done
        while not drip_done and not all(k in emitted for k in keys):
            try:
                next(drip)
            except StopIteration:
                drip_done = True

    # ---- phase B: attention, one flat pipelined stream per pair ----
    # job (pair, qb, t): scores for both heads as concurrent PE row tiles ->
    # one [128,1024] exp -> PV accumulation (pipelined DEPTH back)
    sc_ps = bstack.enter_context(tc.tile_pool(name="sc_ps", bufs=2, space="PSUM"))
    pv_ps = bstack.enter_context(tc.tile_pool(name="pv_ps", bufs=3, space="PSUM"))
    probs_pool = bstack.enter_context(tc.tile_pool(name="probs", bufs=DEPTH + 6))
    bcast_pool = bstack.enter_context(tc.tile_pool(name="bcast", bufs=5))
    y_sb = bstack.enter_context(tc.tile_pool(name="y_sb", bufs=6))

    # output-projection units, dripped into pair-1's attention stream as the
    # attn q-blocks drain (ps slots shared with the projection drip)
    def emit_c_unit(mt, n2, evac=None, pool=None):
        if pool is None:
            pool = qk_ps
        ps = pool.tile([128, 512], FP, name="qps" if pool is qk_ps else "pv")
        for p in range(2):
            nc.tensor.matmul(
                ps,
                attn[p][:, mt * 128 : (mt + 1) * 128],
                wo_sb[p][:, n2 * 512 : (n2 + 1) * 512],
                start=(p == 0),
                stop=(p == 1),
            )
        yt = y_sb.tile([128, 512], BF, name="yt")
        if evac == "scalar":
            nc.scalar.copy(yt, ps)
        else:
            nc.vector.tensor_copy(yt, ps)
        yq = nc.default_dma_engine if (mt + n2) % 2 == 0 else nc.sync
        yq.dma_start(
            out=y[mt * 128 : (mt + 1) * 128, n2 * 512 : (n2 + 1) * 512], in_=yt
        )

    c_units = [(mt, n2) for mt in range(SKT) for n2 in range(2)]
    c_avail = 0
    c_emitted = 0

    jobs = [(pi, qb, t) for pi in range(2) for qb in range(NT) for t in range(SKT)]
    pr_slots = [None] * len(jobs)
    pvs = None
    for i in range(len(jobs) + DEPTH):
        if i < len(jobs):
            pi, qb, t = jobs[i]
            drip_until(("q", 2 + pi, t // 4), ("q", pi, qb))
            s2 = sc_ps.tile([128, 1024], FP, name="s2")
            for j in range(2):  # j=0 -> head off 0 (tile 0,0), j=1 -> off 64
                off = j * 64
                nc.tensor.matmul(
                    s2[:, j * 512 : (j + 1) * 512],
                    kT[pi][off : off + 64, t * 128 : (t + 1) * 128],
                    qT[pi][off : off + 64, qb * 512 : (qb + 1) * 512],
                    start=True,
                    stop=True,
                )
            pr2 = probs_pool.tile([128, 1024], BF, name="pr2")
            nc.scalar.activation(
                pr2, s2, mybir.ActivationFunctionType.Exp, scale=SCALE
            )
            pr_slots[i] = pr2
            # budget ramp: early pair-0 jobs already carry forced just-in-time
            # waves via drip_until, so the extra budget only starts at job 12;
            # pair-1 jobs carry a light budget (C units share their slack)
            if pi == 0:
                drip_steps(100.0 if i < 12 else DRIP_NS)
            else:
                drip_steps(160.0)
        io = i - DEPTH
        if io >= 0:
            pi0, qb0, t0 = jobs[io]
            drip_until(("v", pi0, t0))
            if t0 == 0:
                pvs = [pv_ps.tile([VBLK, 512], FP, name="pv") for j in range(2)]
            pr0 = pr_slots[io]
            pr_slots[io] = None
            for j in range(2):
                h0 = pi0 * 2 + j
                blk = (t0 * HPG + h0) * VBLK
                nc.tensor.matmul(
                    pvs[j],
                    v_store[:, blk : blk + VBLK],
                    pr0[:, j * 512 : (j + 1) * 512],
                    start=(t0 == 0),
                    stop=(t0 == SKT - 1),
                )
            if t0 == SKT - 1:
                if pi0 == 1:
                    c_avail += 8  # out-proj for q-block qb0 now fully drained
                # drain pair: denominators -> reciprocal -> broadcast -> evac
                # (reciprocal_approx_fast must not read PSUM directly: the
                # custom-DVE lowering silently miscompiles on HW)
                r0 = (pi0 * NT + qb0) * 2
                for j in range(2):
                    nc.vector.tensor_copy(
                        den_all[:, (r0 + j) * 512 : (r0 + j + 1) * 512],
                        pvs[j][64:65, :],
                    )
                nc.vector.reciprocal_approx_fast(
                    rden_all[:, r0 * 512 : (r0 + 2) * 512],
                    den_all[:, r0 * 512 : (r0 + 2) * 512],
                )
                rden128 = bcast_pool.tile([128, 1024], FP, name="rb")
                nc.gpsimd.partition_broadcast(
                    rden128, rden_all[:, r0 * 512 : (r0 + 2) * 512]
                )
                for j in range(2):
                    off = j * 64
                    nc.vector.tensor_mul(
                        attn[pi0][off : off + 64, qb0 * 512 : (qb0 + 1) * 512],
                        pvs[j][0:64, :],
                        rden128[off : off + 64, j * 512 : (j + 1) * 512],
                    )
        if c_emitted < c_avail:
            emit_c_unit(*c_units[c_emitted])
            c_emitted += 1

    # flush remaining output-projection units. ACT's exp queue is drained by
    # now, so alternate the evacuation between DVE and ACT; rotate psum slots
    # across the qps slot AND the (same-sized, now-idle) pv ring so units
    # pipeline instead of serializing on one bank
    flush_pools = [qk_ps, pv_ps, pv_ps, pv_ps]
    fi = 0
    while c_emitted < len(c_units):
        emit_c_unit(
            *c_units[c_emitted],
            evac="scalar" if c_emitted % 2 else None,
            pool=flush_pools[fi % 4],
        )
        fi += 1
        c_emitted += 1

    bstack.close()


_PROGRAM = None


def _get_program():
    global _PROGRAM
    if _PROGRAM is None:
        nc = bacc.Bacc(
            "TRN2",
            target_bir_lowering=False,
            debug=False,
            enable_asserts=False,
            num_devices=NCORES,
        )
        xt = nc.dram_tensor("xt", [E, S], BF, kind="ExternalInput").ap()
        w = nc.dram_tensor("wqkv", [E, 768], BF, kind="ExternalInput").ap()
        bq = nc.dram_tensor("bqkv", [768, 1], FP, kind="ExternalInput").ap()
        wo = nc.dram_tensor("wout", [DG, E], BF, kind="ExternalInput").ap()
        y = nc.dram_tensor("y", [S, E], BF, kind="ExternalOutput").ap()
        with tile.TileContext(nc) as tc:
            _mha_body(tc, xt, w, bq, wo, y)
        nc.compile()
        _PROGRAM = nc
    return _PROGRAM


LAST_RESULTS = None


def make_in_map(c, x, Wqkv, bqkv, Wout):
    b, g = divmod(c, GROUPS)
    # reference layout: Wqkv column j -> head j//192, role (j%192)//64
    idx_q = np.concatenate(
        [np.arange(h * 3 * HD, h * 3 * HD + HD)
         for h in range(g * HPG, (g + 1) * HPG)]
    )
    cols = np.concatenate([idx_q, idx_q + HD, idx_q + 2 * HD])
    w_loc = Wqkv[:, cols]
    b_loc = bqkv[cols][:, None]
    cs = slice(g * DG, (g + 1) * DG)
    return {
        "xt": np.ascontiguousarray(x[b].T).astype(ml_dtypes.bfloat16),
        "wqkv": np.ascontiguousarray(w_loc).astype(ml_dtypes.bfloat16),
        "bqkv": np.ascontiguousarray(b_loc),
        "wout": np.ascontiguousarray(Wout[cs, :]).astype(ml_dtypes.bfloat16),
    }


def kernel(x, Wqkv, bqkv, Wout, bout):
    global LAST_RESULTS
    x = np.asarray(x, np.float32)
    Wqkv = np.asarray(Wqkv, np.float32)
    bqkv = np.asarray(bqkv, np.float32)
    Wout = np.asarray(Wout, np.float32)
    bout = np.asarray(bout, np.float32)

    nc = _get_program()
    in_maps = [make_in_map(c, x, Wqkv, bqkv, Wout) for c in range(NCORES)]

    res = run_bass_kernel_spmd(
        nc,
        in_maps,
        core_ids=list(range(NCORES)),
        trace=bool(int(os.environ.get("KERNEL_TRACE", "0"))),
    )
    LAST_RESULTS = res

    out = np.empty((B, S, E), np.float32)
    for b in range(B):
        acc = res.results[b * GROUPS]["y"].astype(np.float32)
        for g in range(1, GROUPS):
            acc += res.results[b * GROUPS + g]["y"].astype(np.float32)
        out[b] = acc + bout[None, :]
    return out


# revision 53
# speedup vs baseline: 1.0347x; 1.0347x over previous
"""Multi-head self-attention on 8 Trainium2 NeuronCores.

Sharding: core c = b*4 + g handles batch b (of 2) and head-group g (4 heads
of 16). Per core: full qkv projection for its 4 heads, attention, and a
partial output projection (row-slice of Wout). Host sums the 4 partials per
batch and adds bout.

The ACT engine is the hard floor (16.8M softmax exps at 1 elem/cycle/lane =
~109us/core), so the kernel keeps ACT fed continuously: only pair-0's
projection runs up front; pair-1's projection matmuls drip between pair-0
attention jobs inside the ACT slack. All matmul operands are bf16 so weight
loads stream via LDWEIGHTS (fp32r stationaries force serial self-loading
matmuls on HW). Scores for the two heads of a pair are issued as PE row
tiles (0,0)/(64,0): contraction is head_dim=64, so both 64-row tiles execute
concurrently, doubling score throughput. One [128,1024] exp per job covers
both heads. PV keeps the ones-column denominator trick; normalization is
fused into the PSUM evacuation (tensor_mul with the broadcast reciprocal).
"""

import os
from contextlib import ExitStack

import ml_dtypes
import numpy as np

import concourse.bass as bass
import concourse.bacc as bacc
import concourse.tile as tile
from concourse import mybir
from concourse._compat import with_exitstack
from concourse.bass_utils import run_bass_kernel_spmd

B, S, E, H = 2, 2048, 1024, 16
HD = 64
SCALE = HD ** -0.5
NCORES = 8
GROUPS = 4                 # head-groups per batch == cores per batch
HPG = H // GROUPS          # 4 heads per core
DG = HPG * HD              # 256 qkv cols per core per projection
KC = E // 128              # 8 contraction chunks
NT = S // 512              # 4 q/token chunks of 512
SKT = S // 128             # 16 key tiles of 128
VBLK = 65                  # v block cols: 64 v dims + ones column
DEPTH = 4                  # attention software-pipeline depth
DRIP_NS = 460.0            # PE-ns of dripped projection work per attention job

FP = mybir.dt.float32
BF = mybir.dt.bfloat16


@with_exitstack
def _mha_body(ctx: ExitStack, tc: tile.TileContext, xt, w, bqkv, wo, y):
    nc = tc.nc
    main = ctx.enter_context(tc.tile_pool(name="main", bufs=1))

    qT = [main.tile([128, S], BF, name=f"qT{p}") for p in range(2)]
    kT = [main.tile([128, S], BF, name=f"kT{p}") for p in range(2)]
    v_store = main.tile([128, SKT * HPG * VBLK], BF)   # [128, 4160]
    attn = [main.tile([128, S], BF, name=f"attn{p}") for p in range(2)]
    wo_sb = [main.tile([128, E], BF, name=f"wo{p}") for p in range(2)]
    b_sb = main.tile([128, 4], FP)
    vb = main.tile([128, DG], FP)
    den_all = main.tile([1, 16 * 512], FP)
    rden_all = main.tile([1, 16 * 512], FP)
    xts = [main.tile([128, S], BF, name=f"xts{k}") for k in range(KC)]
    wts = [main.tile([128, 768], BF, name=f"wts{k}") for k in range(KC)]

    # weights first, then x in token-column blocks: the first projection
    # waves need only token block 0, so compute starts ~3MB earlier. Biases
    # load after the w chunks (they aren't read until the first wave's evac).
    for k in range(KC):
        wq = nc.sync if k % 2 == 0 else nc.gpsimd
        wq.dma_start(out=wts[k], in_=w[k * 128 : (k + 1) * 128, :])
    for m in range(4):
        nc.gpsimd.dma_start(out=b_sb[:, m : m + 1], in_=bqkv[m * 128 : (m + 1) * 128, :])
    vb_row = main.tile([1, DG], FP)
    nc.gpsimd.dma_start(out=vb_row, in_=bqkv[512:768, :])
    nc.gpsimd.partition_broadcast(vb, vb_row)
    for nb in range(NT):
        for k in range(KC):
            xq = nc.default_dma_engine if k % 2 == 0 else nc.scalar
            xq.dma_start(
                out=xts[k][:, nb * 512 : (nb + 1) * 512],
                in_=xt[k * 128 : (k + 1) * 128, nb * 512 : (nb + 1) * 512],
            )
    for p in range(2):
        nc.gpsimd.dma_start(out=wo_sb[p], in_=wo[p * 128 : (p + 1) * 128, :])

    vs_view = v_store.rearrange("p (j c) -> p j c", c=VBLK)
    ones_src = main.tile([128, SKT * HPG], BF)
    nc.vector.memset(ones_src, 1.0)
    nc.vector.tensor_copy(
        vs_view[:, :, 64:65], ones_src.rearrange("p (j c) -> p j c", c=1)
    )
    # dummy exp: pulls the ~2.7us ACT_TABLE_LOAD into the DMA window
    act_warm = main.tile([1, 1], FP)
    nc.scalar.activation(
        act_warm, ones_src[0:1, 0:1], mybir.ActivationFunctionType.Exp, scale=1.0
    )

    # m: 0=qT0 (heads 0,1), 1=qT1, 2=kT0, 3=kT1
    dsts = [qT[0], qT[1], kT[0], kT[1]]
    bstack = ctx.enter_context(ExitStack())
    qk_ps = bstack.enter_context(tc.tile_pool(name="qk_ps", bufs=1, space="PSUM"))
    v_ps = qk_ps
    sc_ps = bstack.enter_context(tc.tile_pool(name="sc_ps", bufs=2, space="PSUM"))
    pv_ps = bstack.enter_context(tc.tile_pool(name="pv_ps", bufs=3, space="PSUM"))
    probs_pool = bstack.enter_context(tc.tile_pool(name="probs", bufs=DEPTH + 6))
    bcast_pool = bstack.enter_context(tc.tile_pool(name="bcast", bufs=5))
    y_sb = bstack.enter_context(tc.tile_pool(name="y_sb", bufs=6))

    # projection work units: ("q", m, n) -> one [128,512] q/k block;
    # ("v", pair, tt) -> one [128,128] v token-tile for the pair's 2 heads
    emitted = set()

    def emit_q_wave(m, n, yielding, pool=None):
        if pool is None:
            pool = qk_ps
        ps = pool.tile([128, 512], FP, name="qps" if pool is qk_ps else "pv")
        for k in range(KC):
            nc.tensor.matmul(
                ps,
                wts[k][:, m * 128 : (m + 1) * 128],
                xts[k][:, n * 512 : (n + 1) * 512],
                start=(k == 0),
                stop=(k == KC - 1),
            )
            if yielding:
                yield 213.0
        nc.vector.tensor_scalar_add(
            dsts[m][:, n * 512 : (n + 1) * 512], ps, b_sb[:, m : m + 1]
        )
        emitted.add(("q", m, n))
        if yielding:
            yield 0.0

    def emit_v_wave(pair, tt, yielding):
        # shares the qps PSUM slot (only cols 0:128 used) to save a bank
        vp = v_ps.tile([128, 512], FP, name="qps")[:, 0:128]
        for k in range(KC):
            nc.tensor.matmul(
                vp,
                xts[k][:, tt * 128 : (tt + 1) * 128],
                wts[k][:, 512 + pair * 128 : 640 + pair * 128],
                start=(k == 0),
                stop=(k == KC - 1),
            )
            if yielding:
                yield 80.0
        nc.vector.tensor_add(
            vs_view[:, tt * HPG + 2 * pair : tt * HPG + 2 * pair + 2, 0:64],
            vp.rearrange("p (j c) -> p j c", c=64),
            vb[:, pair * 128 : pair * 128 + 128].rearrange("p (j c) -> p j c", c=64),
        )
        emitted.add(("v", pair, tt))
        if yielding:
            yield 0.0

    # ---- phase A0: minimal prefix — kT0 keys 0:512, qT0 block 0, v0 tile 0.
    # qT0 borrows an idle pv slot so the waves don't serialize on qps evacs.
    for _ in emit_q_wave(2, 0, False):
        pass
    for _ in emit_q_wave(0, 0, False, pool=pv_ps):
        pass
    for _ in emit_v_wave(0, 0, False):
        pass

    # everything else drips between attention jobs, ordered so each unit
    # lands ahead of its consumer (deadlines in B0/B1 job indices; the
    # drip_until gates force just-in-time emission if the budget lags)
    drip_items = (
        [("q", 2, 1), ("v", 0, 1), ("v", 0, 2), ("v", 0, 3)]
        + [("q", 2, 2), ("v", 0, 4), ("v", 0, 5), ("v", 0, 6)]
        + [("q", 2, 3), ("v", 0, 7), ("v", 0, 8), ("v", 0, 9)]
        + [("q", 0, 1)]
        + [("v", 0, tt) for tt in (10, 11, 12)]
        + [("q", 0, 2)]
        + [("v", 0, tt) for tt in (13, 14, 15)]
        + [("q", 0, 3)]
        + [("q", 3, n) for n in range(NT)]
        + [("q", 1, 0)]
        + [("v", 1, tt) for tt in range(4)]
        + [("q", 1, 1)]
        + [("v", 1, tt) for tt in range(4, 9)]
        + [("q", 1, 2)]
        + [("v", 1, tt) for tt in range(9, 13)]
        + [("q", 1, 3)]
        + [("v", 1, tt) for tt in range(13, 16)]
    )

    def drip_gen():
        for item in drip_items:
            if item[0] == "q":
                yield from emit_q_wave(item[1], item[2], True)
            else:
                yield from emit_v_wave(item[1], item[2], True)

    drip = drip_gen()
    drip_done = False
    drip_budget = 0.0

    def drip_steps(ns):
        nonlocal drip_budget, drip_done
        drip_budget += ns
        while not drip_done and drip_budget > 0:
            try:
                drip_budget -= next(drip)
            except StopIteration:
                drip_done = True

    def drip_until(*keys):
        # program-order deadline: a consumer on the PE queue must be emitted
        # after its producers; force-drain the drip to the needed unit
        nonlocal drip_done
        while not drip_done and not all(k in emitted for k in keys):
            try:
                next(drip)
            except StopIteration:
                drip_done = True

    # ---- phase B: attention, one flat pipelined stream per pair ----
    # job (pair, qb, t): scores for both heads as concurrent PE row tiles ->
    # one [128,1024] exp -> PV accumulation (pipelined DEPTH back)
    # output-projection units, dripped into pair-1's attention stream as the
    # attn q-blocks drain (ps slots shared with the projection drip)
    def emit_c_unit(mt, n2, evac=None, pool=None):
        if pool is None:
            pool = qk_ps
        ps = pool.tile([128, 512], FP, name="qps" if pool is qk_ps else "pv")
        for p in range(2):
            nc.tensor.matmul(
                ps,
                attn[p][:, mt * 128 : (mt + 1) * 128],
                wo_sb[p][:, n2 * 512 : (n2 + 1) * 512],
                start=(p == 0),
                stop=(p == 1),
            )
        yt = y_sb.tile([128, 512], BF, name="yt")
        if evac == "scalar":
            nc.scalar.copy(yt, ps)
        else:
            nc.vector.tensor_copy(yt, ps)
        yq = nc.default_dma_engine if (mt + n2) % 2 == 0 else nc.sync
        yq.dma_start(
            out=y[mt * 128 : (mt + 1) * 128, n2 * 512 : (n2 + 1) * 512], in_=yt
        )

    c_units = [(mt, n2) for mt in range(SKT) for n2 in range(2)]
    c_avail = 0
    c_emitted = 0

    jobs = [(pi, qb, t) for pi in range(2) for qb in range(NT) for t in range(SKT)]
    pr_slots = [None] * len(jobs)
    pvs = None
    for i in range(len(jobs) + DEPTH):
        if i < len(jobs):
            pi, qb, t = jobs[i]
            drip_until(("q", 2 + pi, t // 4), ("q", pi, qb))
            s2 = sc_ps.tile([128, 1024], FP, name="s2")
            for j in range(2):  # j=0 -> head off 0 (tile 0,0), j=1 -> off 64
                off = j * 64
                nc.tensor.matmul(
                    s2[:, j * 512 : (j + 1) * 512],
                    kT[pi][off : off + 64, t * 128 : (t + 1) * 128],
                    qT[pi][off : off + 64, qb * 512 : (qb + 1) * 512],
                    start=True,
                    stop=True,
                )
            pr2 = probs_pool.tile([128, 1024], BF, name="pr2")
            nc.scalar.activation(
                pr2, s2, mybir.ActivationFunctionType.Exp, scale=SCALE
            )
            pr_slots[i] = pr2
            # budget ramp: early pair-0 jobs already carry forced just-in-time
            # waves via drip_until, so the extra budget only starts at job 12;
            # pair-1 jobs carry a light budget (C units share their slack)
            if pi == 0:
                drip_steps(100.0 if i < 12 else DRIP_NS)
            else:
                drip_steps(160.0)
        io = i - DEPTH
        if io >= 0:
            pi0, qb0, t0 = jobs[io]
            drip_until(("v", pi0, t0))
            if t0 == 0:
                pvs = [pv_ps.tile([VBLK, 512], FP, name="pv") for j in range(2)]
            pr0 = pr_slots[io]
            pr_slots[io] = None
            for j in range(2):
                h0 = pi0 * 2 + j
                blk = (t0 * HPG + h0) * VBLK
                nc.tensor.matmul(
                    pvs[j],
                    v_store[:, blk : blk + VBLK],
                    pr0[:, j * 512 : (j + 1) * 512],
                    start=(t0 == 0),
                    stop=(t0 == SKT - 1),
                )
            if t0 == SKT - 1:
                if pi0 == 1:
                    c_avail += 8  # out-proj for q-block qb0 now fully drained
                # drain pair: denominators -> reciprocal -> broadcast -> evac
                # (reciprocal_approx_fast must not read PSUM directly: the
                # custom-DVE lowering silently miscompiles on HW)
                r0 = (pi0 * NT + qb0) * 2
                for j in range(2):
                    nc.vector.tensor_copy(
                        den_all[:, (r0 + j) * 512 : (r0 + j + 1) * 512],
                        pvs[j][64:65, :],
                    )
                nc.vector.reciprocal_approx_fast(
                    rden_all[:, r0 * 512 : (r0 + 2) * 512],
                    den_all[:, r0 * 512 : (r0 + 2) * 512],
                )
                rden128 = bcast_pool.tile([128, 1024], FP, name="rb")
                nc.gpsimd.partition_broadcast(
                    rden128, rden_all[:, r0 * 512 : (r0 + 2) * 512]
                )
                for j in range(2):
                    off = j * 64
                    nc.vector.tensor_mul(
                        attn[pi0][off : off + 64, qb0 * 512 : (qb0 + 1) * 512],
                        pvs[j][0:64, :],
                        rden128[off : off + 64, j * 512 : (j + 1) * 512],
                    )
        if c_emitted < c_avail:
            emit_c_unit(*c_units[c_emitted])
            c_emitted += 1

    # flush remaining output-projection units. ACT's exp queue is drained by
    # now, so alternate the evacuation between DVE and ACT; rotate psum slots
    # across the qps slot AND the (same-sized, now-idle) pv ring so units
    # pipeline instead of serializing on one bank
    flush_pools = [qk_ps, pv_ps, pv_ps, pv_ps]
    fi = 0
    while c_emitted < len(c_units):
        emit_c_unit(
            *c_units[c_emitted],
            evac="scalar" if c_emitted % 2 else None,
            pool=flush_pools[fi % 4],
        )
        fi += 1
        c_emitted += 1

    bstack.close()


_PROGRAM = None


def _get_program():
    global _PROGRAM
    if _PROGRAM is None:
        nc = bacc.Bacc(
            "TRN2",
            target_bir_lowering=False,
            debug=False,
            enable_asserts=False,
            num_devices=NCORES,
        )
        xt = nc.dram_tensor("xt", [E, S], BF, kind="ExternalInput").ap()
        w = nc.dram_tensor("wqkv", [E, 768], BF, kind="ExternalInput").ap()
        bq = nc.dram_tensor("bqkv", [768, 1], FP, kind="ExternalInput").ap()
        wo = nc.dram_tensor("wout", [DG, E], BF, kind="ExternalInput").ap()
        y = nc.dram_tensor("y", [S, E], BF, kind="ExternalOutput").ap()
        with tile.TileContext(nc) as tc:
            _mha_body(tc, xt, w, bq, wo, y)
        nc.compile()
        _PROGRAM = nc
    return _PROGRAM


LAST_RESULTS = None


def make_in_map(c, x, Wqkv, bqkv, Wout):
    b, g = divmod(c, GROUPS)
    # reference layout: Wqkv column j -> head j//192, role (j%192)//64
    idx_q = np.concatenate(
        [np.arange(h * 3 * HD, h * 3 * HD + HD)
         for h in range(g * HPG, (g + 1) * HPG)]
    )
    cols = np.concatenate([idx_q, idx_q + HD, idx_q + 2 * HD])
    w_loc = Wqkv[:, cols]
    b_loc = bqkv[cols][:, None]
    cs = slice(g * DG, (g + 1) * DG)
    return {
        "xt": np.ascontiguousarray(x[b].T).astype(ml_dtypes.bfloat16),
        "wqkv": np.ascontiguousarray(w_loc).astype(ml_dtypes.bfloat16),
        "bqkv": np.ascontiguousarray(b_loc),
        "wout": np.ascontiguousarray(Wout[cs, :]).astype(ml_dtypes.bfloat16),
    }


def kernel(x, Wqkv, bqkv, Wout, bout):
    global LAST_RESULTS
    x = np.asarray(x, np.float32)
    Wqkv = np.asarray(Wqkv, np.float32)
    bqkv = np.asarray(bqkv, np.float32)
    Wout = np.asarray(Wout, np.float32)
    bout = np.asarray(bout, np.float32)

    nc = _get_program()
    in_maps = [make_in_map(c, x, Wqkv, bqkv, Wout) for c in range(NCORES)]

    res = run_bass_kernel_spmd(
        nc,
        in_maps,
        core_ids=list(range(NCORES)),
        trace=bool(int(os.environ.get("KERNEL_TRACE", "0"))),
    )
    LAST_RESULTS = res

    out = np.empty((B, S, E), np.float32)
    for b in range(B):
        acc = res.results[b * GROUPS]["y"].astype(np.float32)
        for g in range(1, GROUPS):
            acc += res.results[b * GROUPS + g]["y"].astype(np.float32)
        out[b] = acc + bout[None, :]
    return out
